# revision 1
# baseline (speedup 1.0000x reference)
"""GCN encoder (2-layer GCNConv) on 8 Trainium2 NeuronCores.

Strategy (pull model, dst-sharded):
  out = A @ relu(A @ x @ W1 + b1) @ W2 + b2,  A = D^-1/2 (Adj+I) D^-1/2
Reassociate: agg = A @ x first, then dense matmul by W (A@(xW) == (A@x)W).
Fold the src-side dinv into x on the host (x~ = dinv * x) and the dst-side
dinv into a per-partition ACT scale.  The sparse aggregation is done as
PE selection-matrix matmuls over edge chunks of 128:
  psum[feat, dst128] += M_chunk[e,feat].T @ S_chunk[e, dst128]
where M_chunk is dma_gather'ed rows of x~ (bf16) and S is a 0/1 matrix
built on DVE with is_equal(iota, dstl).

Host-side: nodes are dealt into 784 tiles of 128 slots (degree-stratified
round robin), edges are grouped by (dst tile, src bank) where banks are 6
overlapping 32768-row windows of the slot space (dma_gather indices are
int16).  Each (tile, bank) cell is padded to exactly 384 edge slots so one
static NEFF serves all 8 cores.  h1 is exchanged with an AllGather.
"""

import os

import numpy as np
import ml_dtypes

# ---------------------------------------------------------------- constants
N_NODES = 100000
N_EDGES = 1600000
IN_DIM = 128
HID_DIM = 128
OUT_DIM = 64
P = 128

N_CORES = 8
TPC = 98                    # tiles per core
SPC = TPC * P               # 12544 slots per core
NS = N_CORES * SPC          # 100352 slots total
NT = N_CORES * TPC          # 784 tiles total

N_BANKS = 6
BANK_ROWS = 32768
BANK_OFF = [0, 13517, 27034, 40551, 54068, 67584]
CPB = 3                     # chunks per (tile, bank)
SPTB = CPB * P              # 384 edge slots per (tile, bank)
CPT = N_BANKS * CPB         # 18 chunks per tile
SPT = CPT * P               # 2304 edge slots per tile
GROUP_TILES = 7
N_GROUPS = TPC // GROUP_TILES   # 14
SEG_IDXS = GROUP_TILES * SPTB   # 2688 idxs per (group, bank) gather
SEG_COLS = SEG_IDXS // 16       # 168
N_SEGS = N_GROUPS * N_BANKS     # 84
IDX_COLS = N_SEGS * SEG_COLS    # 14112

BF16 = ml_dtypes.bfloat16

LAST_RESULTS = None


# ================================================================ host prep
def _preprocess(x, edge_index):
    x = np.asarray(x, dtype=np.float32)
    ei = np.asarray(edge_index, dtype=np.int64)
    src = ei[0]
    dst = ei[1]

    # degree includes the self loop (appended by the reference)
    deg = (np.bincount(dst, minlength=N_NODES) + 1).astype(np.float64)
    dinv = 1.0 / np.sqrt(np.maximum(deg, 1e-12))

    # Self loops are NOT routed through the gather: their contribution is a
    # diagonal term handled by one identity-rhs matmul per tile.

    # ---- slot assignment: degree-stratified round robin (snake) over tiles
    order = np.argsort(-deg, kind="stable")
    k = np.arange(N_NODES)
    r = k // NT
    pos = k % NT
    tile_of_k = np.where(r % 2 == 0, pos, NT - 1 - pos)
    slot_of_node = np.empty(N_NODES, dtype=np.int64)
    slot_of_node[order] = tile_of_k * P + r

    # per-tile degree check
    tile_deg = np.bincount(slot_of_node[dst] // P, minlength=NT)
    assert tile_deg.max() <= SPT - 24, f"tile overload: {tile_deg.max()}"

    # ---- per-edge quantities
    eslot_dst = slot_of_node[dst]
    tile_e = (eslot_dst // P).astype(np.int64)
    dstl_e = (eslot_dst % P).astype(np.int32)
    sslot = slot_of_node[src].astype(np.int64)

    offs = np.asarray(BANK_OFF, dtype=np.int64)
    # allowed banks for edge e: lo_e..hi_e  (interval)
    lo_e = np.searchsorted(offs, sslot - (BANK_ROWS - 1), side="left")
    hi_e = np.searchsorted(offs, sslot, side="right") - 1
    assert (lo_e <= hi_e).all()

    # ---- per-tile EDF bank assignment with caps of SPTB real edges
    order_e = np.lexsort((hi_e, tile_e))   # by tile, then deadline
    t_sorted = tile_e[order_e]
    tile_starts = np.searchsorted(t_sorted, np.arange(NT + 1))

    # outputs
    gidx = np.zeros((N_CORES, P, IDX_COLS), dtype=np.int16)
    dstp = np.full((N_CORES, P, TPC * CPT), 200.0, dtype=np.float32)
    rng = np.random.RandomState(1234)

    flat_idx = np.empty((N_CORES, TPC, N_BANKS, SPTB), dtype=np.int16)
    flat_dstl = np.full((N_CORES, TPC, N_BANKS, SPTB), 200, dtype=np.int32)

    for t in range(NT):
        es = order_e[tile_starts[t]:tile_starts[t + 1]]   # edges, by hi asc
        elo = lo_e[es]
        ehi = hi_e[es]
        assigned = np.full(len(es), -1, dtype=np.int8)
        for b in range(N_BANKS):
            cand = np.nonzero((assigned == -1) & (elo <= b))[0]
            take = cand[:SPTB]
            assigned[take] = b
            left = (assigned == -1) & (ehi == b)
            if left.any():
                raise RuntimeError(f"bank overflow tile {t} bank {b}")
        assert (assigned >= 0).all()
        c = t // TPC
        tl = t % TPC
        for b in range(N_BANKS):
            sel = es[assigned == b]
            n = len(sel)
            assert n <= SPTB
            fi = flat_idx[c, tl, b]
            fd = flat_dstl[c, tl, b]
            fi[:n] = (sslot[sel] - BANK_OFF[b]).astype(np.int16)
            fd[:n] = dstl_e[sel]
            if n < SPTB:
                fi[n:] = rng.randint(0, BANK_ROWS, size=SPTB - n).astype(np.int16)
                # fd stays 200 (pad -> S row all zero)

    # sanity: idx in range
    assert flat_idx.min() >= 0

    # ---- pack gidx (wrapped 16, replicated to 128 partitions) and dstp
    for c in range(N_CORES):
        for g in range(N_GROUPS):
            for b in range(N_BANKS):
                seg = g * N_BANKS + b
                # concat the 7 tiles' (t,b) runs
                vals = flat_idx[c, g * GROUP_TILES:(g + 1) * GROUP_TILES, b].reshape(-1)
                w = vals.reshape(SEG_COLS, 16).T          # [16, SEG_COLS]
                gidx[c, :, seg * SEG_COLS:(seg + 1) * SEG_COLS] = np.tile(w, (8, 1))
        # dstp: col = tl*CPT + b*CPB + jc ; partition = j%128
        d = flat_dstl[c].reshape(TPC, N_BANKS, CPB, P)    # [tl, b, jc, p]
        d = d.transpose(3, 0, 1, 2).reshape(P, TPC * CPT)
        dstp[c] = d.astype(np.float32)

    # ---- node-feature table in slot order, pre-scaled by dinv (bf16)
    xt = np.zeros((NS, IN_DIM), dtype=BF16)
    xt[slot_of_node] = (x * dinv[:, None].astype(np.float32)).astype(BF16)

    # ---- per-core dinv (ACT scale) and rdinv (bias rank-1 lhsT)
    dinv_slots = np.zeros(NS, dtype=np.float32)
    dinv_slots[slot_of_node] = dinv.astype(np.float32)
    rdinv_slots = np.zeros(NS, dtype=np.float32)
    rdinv_slots[slot_of_node] = (1.0 / dinv).astype(np.float32)
    dinv_t = dinv_slots.reshape(N_CORES, TPC, P).transpose(0, 2, 1).copy()  # [c,128,98]
    # layer-1 ACT scale is dinv^2: it also folds the src-side dinv the
    # layer-2 gather needs into the h1 table (relu commutes with scale>0)
    dinv2_t = (dinv_t * dinv_t).astype(np.float32)
    rdinv_row = rdinv_slots.reshape(N_CORES, 1, SPC).astype(BF16)           # [c,1,12544]

    iota = np.tile(np.arange(P, dtype=np.float32).astype(BF16)[None, :], (P, 1))
    ident = np.eye(P, dtype=np.float32).astype(BF16)

    return dict(
        gidx=gidx, dstp=dstp, xt=xt, dinv_t=dinv_t, dinv2_t=dinv2_t,
        rdinv_row=rdinv_row, iota=iota, ident=ident,
        slot_of_node=slot_of_node,
    )


# ============================================================ numpy emulator
def _emulate(prep, W1, b1, W2, b2):
    """Numpy bit-for-bit-ish emulation of the device kernel (fp32 math on
    bf16-rounded data) to validate all the host-side layout logic."""
    xt = prep["xt"].astype(np.float32)
    gidx = prep["gidx"]
    dstp = prep["dstp"].astype(np.float32)
    dinv_t = prep["dinv_t"]
    rdinv = prep["rdinv_row"].astype(np.float32)
    w1 = W1.astype(BF16).astype(np.float32)
    w2 = W2.astype(BF16).astype(np.float32)
    b1f = b1.astype(BF16).astype(np.float32)
    b2f = b2.astype(BF16).astype(np.float32)

    def unwrap_seg(c, seg):
        w = gidx[c, :16, seg * SEG_COLS:(seg + 1) * SEG_COLS]
        return w.T.reshape(-1)   # [2688]

    def layer(table, w, bvec, relu, out_dim, scale_t):
        # table [NS, F] fp32 (already bf16-rounded values)
        h_out = np.zeros((N_CORES, SPC, out_dim), dtype=np.float32)
        F = table.shape[1]
        for c in range(N_CORES):
            for g in range(N_GROUPS):
                M = np.zeros((N_BANKS, GROUP_TILES * CPB, P, F), np.float32)
                for b in range(N_BANKS):
                    idxs = unwrap_seg(c, g * N_BANKS + b)
                    rows = table[BANK_OFF[b] + idxs.astype(np.int64)]
                    M[b] = rows.reshape(GROUP_TILES * CPB, P, F)
                for ti in range(GROUP_TILES):
                    tl = g * GROUP_TILES + ti
                    base = c * SPC + tl * P
                    # self-loop diagonal: psum[:, d] += table[base + d]
                    psum = table[base:base + P].astype(BF16).astype(np.float32).T.copy()
                    for cch in range(CPT):
                        b, j = divmod(cch, CPB)
                        mc = M[b, ti * CPB + j]            # [128e, F]
                        dcol = dstp[c, :, tl * CPT + cch]  # [128]
                        S = (dcol[:, None] == np.arange(P)[None, :]).astype(np.float32)
                        psum += mc.astype(BF16).astype(np.float32).T @ S
                    aggT = psum.astype(BF16).astype(np.float32)   # [F, 128d]
                    ps_b = aggT.T @ w                              # [128d, out]
                    u = rdinv[c, 0, tl * P:(tl + 1) * P]
                    ps_b = ps_b + u[:, None] * bvec[None, :]
                    scale = scale_t[c, :, tl]
                    o = ps_b * scale[:, None]
                    if relu:
                        o = np.maximum(o, 0.0)
                    h_out[c, tl * P:(tl + 1) * P] = o
        return h_out

    h1 = layer(xt, w1, b1f, True, HID_DIM, prep["dinv2_t"])
    h1_full = h1.reshape(NS, HID_DIM).astype(BF16).astype(np.float32)
    out = layer(h1_full, w2, b2f, False, OUT_DIM, dinv_t)
    return out.reshape(NS, OUT_DIM)[prep["slot_of_node"]]


# ============================================================= bass kernel
# The axon terminal cannot run ncfw collectives (NRT_EXEC_UNIT_UNRECOVERABLE),
# so the two GCN layers run as two NEFFs with a host-side h1 allgather.
_CACHED = {}


def _build_layer_nc(layer, reps=1):
    key = (layer, reps)
    if key in _CACHED:
        return _CACHED[key]

    import concourse.mybir as mybir
    import concourse.tile as tile
    from concourse import bacc, library_config

    f32 = mybir.dt.float32
    bf16 = mybir.dt.bfloat16
    i16 = mybir.dt.int16

    fdim = IN_DIM if layer == 1 else HID_DIM
    odim = HID_DIM if layer == 1 else OUT_DIM
    relu = layer == 1
    out_dt_np = BF16 if layer == 1 else np.float32

    nc = bacc.Bacc("TRN2", target_bir_lowering=False, debug=False,
                   num_devices=N_CORES, name=f"gcn_l{layer}r{reps}")

    tab_d = nc.dram_tensor("tab", [NS, fdim], bf16, kind="ExternalInput")
    self_d = nc.dram_tensor("selfb", [SPC, fdim], bf16, kind="ExternalInput")
    ident_d = nc.dram_tensor("ident", [P, P], bf16, kind="ExternalInput")
    gidx_d = nc.dram_tensor("gidx", [P, IDX_COLS], i16, kind="ExternalInput")
    dstp_d = nc.dram_tensor("dstp", [P, TPC * CPT], f32, kind="ExternalInput")
    dinv_d = nc.dram_tensor("dinv", [P, TPC], f32, kind="ExternalInput")
    rdinv_d = nc.dram_tensor("rdinv", [1, SPC], bf16, kind="ExternalInput")
    iota_d = nc.dram_tensor("iota", [P, P], bf16, kind="ExternalInput")
    w_d = nc.dram_tensor("w", [fdim, odim], bf16, kind="ExternalInput")
    b_d = nc.dram_tensor("b", [1, odim], bf16, kind="ExternalInput")
    out_d = nc.dram_tensor(
        "out", [SPC, odim],
        bf16 if layer == 1 else f32, kind="ExternalOutput")

    GC = GROUP_TILES * CPB          # chunks per bank region in a group (21)
    NCH = N_BANKS * GC              # chunks per group (126)
    ofunc = (mybir.ActivationFunctionType.Relu if relu
             else mybir.ActivationFunctionType.Copy)
    out_sb_dt = bf16 if layer == 1 else f32

    with tile.TileContext(nc) as tc:
        nc.gpsimd.load_library(library_config.mlp)

        with (
            tc.tile_pool(name="const", bufs=1) as constp,
            tc.tile_pool(name="mbuf", bufs=2) as mpool,
            tc.tile_pool(name="sbuf_s", bufs=2) as spool,
            tc.tile_pool(name="agg", bufs=3) as aggp,
            tc.tile_pool(name="outp", bufs=3) as outp,
            tc.tile_pool(name="psA", bufs=2, space="PSUM") as psA,
            tc.tile_pool(name="psB", bufs=2, space="PSUM") as psB,
        ):
            # ---- load constants
            gidx_sb = constp.tile([P, IDX_COLS], i16)
            nc.sync.dma_start(gidx_sb[:], gidx_d[:, :])
            dstp_sb = constp.tile([P, TPC * CPT], f32)
            nc.sync.dma_start(dstp_sb[:], dstp_d[:, :])
            dinv_sb = constp.tile([P, TPC], f32)
            nc.sync.dma_start(dinv_sb[:], dinv_d[:, :])
            rdinv_sb = constp.tile([1, SPC], bf16)
            nc.sync.dma_start(rdinv_sb[:], rdinv_d[:, :])
            iota_sb = constp.tile([P, P], bf16)
            nc.sync.dma_start(iota_sb[:], iota_d[:, :])
            ident_sb = constp.tile([P, P], bf16)
            nc.sync.dma_start(ident_sb[:], ident_d[:, :])
            w_sb = constp.tile([fdim, odim], bf16)
            nc.sync.dma_start(w_sb[:], w_d[:, :])
            b_sb = constp.tile([1, odim], bf16)
            nc.sync.dma_start(b_sb[:], b_d[:, :])

            for g in [gg for _ in range(reps) for gg in range(N_GROUPS)]:
                m_t = mpool.tile([P, NCH, fdim], bf16, tag="m")
                for b in range(N_BANKS):
                    seg = g * N_BANKS + b
                    nc.gpsimd.dma_gather(
                        out_ap=m_t[:, b * GC:(b + 1) * GC, :],
                        in_ap=tab_d[BANK_OFF[b]:BANK_OFF[b] + BANK_ROWS, :],
                        idxs_ap=gidx_sb[:, seg * SEG_COLS:(seg + 1) * SEG_COLS],
                        num_idxs=SEG_IDXS,
                        num_idxs_reg=SEG_IDXS,
                        elem_size=fdim,
                        single_packet=False,
                    )
                # contiguous block of this core's own rows (self loops)
                self_t = mpool.tile([P, GROUP_TILES, fdim], bf16, tag="self")
                nc.sync.dma_start(
                    self_t[:],
                    self_d[g * GROUP_TILES * P:(g + 1) * GROUP_TILES * P, :]
                    .rearrange("(t j) f -> j t f", j=P),
                )
                for ti in range(GROUP_TILES):
                    tl = g * GROUP_TILES + ti
                    s_t = spool.tile([P, CPT, P], bf16, tag="s")
                    for cch in range(CPT):
                        col = tl * CPT + cch
                        nc.vector.tensor_scalar(
                            s_t[:, cch, :], iota_sb[:],
                            dstp_sb[:, col:col + 1], None,
                            mybir.AluOpType.is_equal,
                        )
                    ps_a = psA.tile([P, P], f32, tag="psa")
                    nc.tensor.matmul(
                        ps_a[:], lhsT=self_t[:, ti, :], rhs=ident_sb[:],
                        start=True, stop=False)
                    for cch in range(CPT):
                        b, j = divmod(cch, CPB)
                        nc.tensor.matmul(
                            ps_a[:],
                            lhsT=m_t[:, b * GC + ti * CPB + j, :],
                            rhs=s_t[:, cch, :],
                            start=False, stop=(cch == CPT - 1),
                        )
                    aggT = aggp.tile([P, P], bf16, tag="agg")
                    nc.vector.tensor_copy(aggT[:], ps_a[:])
                    ps_b = psB.tile([P, odim], f32, tag="psb")
                    nc.tensor.matmul(
                        ps_b[:], lhsT=rdinv_sb[:, tl * P:(tl + 1) * P],
                        rhs=b_sb[:], start=True, stop=False)
                    nc.tensor.matmul(
                        ps_b[:], lhsT=aggT[:], rhs=w_sb[:],
                        start=False, stop=True)
                    o_t = outp.tile([P, odim], out_sb_dt, tag="o")
                    nc.scalar.activation(
                        o_t[:], ps_b[:], ofunc,
                        scale=dinv_sb[:, tl:tl + 1])
                    nc.sync.dma_start(
                        out_d[tl * P:(tl + 1) * P, :], o_t[:])

    nc.compile()
    _CACHED[key] = nc
    return nc


# ================================================================== kernel
def _run_layer(layer, table, W, b, prep, trace):
    from concourse.bass_utils import run_bass_kernel_spmd

    nc = _build_layer_nc(layer)
    base = {
        "tab": np.ascontiguousarray(table),
        "iota": np.ascontiguousarray(prep["iota"]),
        "ident": np.ascontiguousarray(prep["ident"]),
        "w": np.ascontiguousarray(np.asarray(W, np.float32).astype(BF16)),
        "b": np.ascontiguousarray(np.asarray(b, np.float32).astype(BF16)[None, :]),
    }
    in_maps = []
    for c in range(N_CORES):
        m = dict(base)
        m["selfb"] = np.ascontiguousarray(table[c * SPC:(c + 1) * SPC])
        m["gidx"] = np.ascontiguousarray(prep["gidx"][c])
        m["dstp"] = np.ascontiguousarray(prep["dstp"][c])
        m["dinv"] = np.ascontiguousarray(
            prep["dinv2_t"][c] if layer == 1 else prep["dinv_t"][c])
        m["rdinv"] = np.ascontiguousarray(prep["rdinv_row"][c])
        in_maps.append(m)
    res = run_bass_kernel_spmd(nc, in_maps, core_ids=list(range(N_CORES)),
                               trace=trace)
    return res, np.concatenate([r["out"] for r in res.results], axis=0)


def kernel(x, edge_index, W1, b1, W2, b2):
    prep = _preprocess(x, edge_index)
    trace = bool(os.environ.get("GCN_TRACE"))

    res1, h1full = _run_layer(1, prep["xt"], W1, b1, prep, trace)
    res2, big = _run_layer(2, h1full, W2, b2, prep, trace)

    global LAST_RESULTS
    LAST_RESULTS = (res1, res2)
    return np.ascontiguousarray(big[prep["slot_of_node"]]).astype(np.float32)



# revision 4
# speedup vs baseline: 2.2772x; 2.2772x over previous
"""GCN encoder (2-layer GCNConv) on 8 Trainium2 NeuronCores.

Strategy (pull model, dst-sharded):
  out = A @ relu(A @ x @ W1 + b1) @ W2 + b2,  A = D^-1/2 (Adj+I) D^-1/2
Reassociate: agg = A @ x first, then dense matmul by W (A@(xW) == (A@x)W).
Fold the src-side dinv into x on the host (x~ = dinv * x) and the dst-side
dinv into a per-partition ACT scale.  The sparse aggregation is done as
PE selection-matrix matmuls over edge chunks of 128:
  psum[feat, dst128] += M_chunk[e,feat].T @ S_chunk[e, dst128]
where M_chunk is dma_gather'ed rows of x~ (bf16) and S is a 0/1 matrix
built on DVE with is_equal(iota, dstl).

Host-side: nodes are dealt into 784 tiles of 128 slots (degree-stratified
round robin), edges are grouped by (dst tile, src bank) where banks are 6
overlapping 32768-row windows of the slot space (dma_gather indices are
int16).  Each (tile, bank) cell is padded to exactly 384 edge slots so one
static NEFF serves all 8 cores.  h1 is exchanged with an AllGather.
"""

import os

import numpy as np
import ml_dtypes

# ---------------------------------------------------------------- constants
N_NODES = 100000
N_EDGES = 1600000
IN_DIM = 128
HID_DIM = 128
OUT_DIM = 64
P = 128

N_CORES = 8
TPC = 98                    # tiles per core
SPC = TPC * P               # 12544 slots per core
NS = N_CORES * SPC          # 100352 slots total
NT = N_CORES * TPC          # 784 tiles total

N_BANKS = 6
BANK_ROWS = 32768
BANK_OFF = [0, 13517, 27034, 40551, 54068, 67584]
CPB = 3                     # chunks per (tile, bank)
SPTB = CPB * P              # 384 edge slots per (tile, bank)
CPT = N_BANKS * CPB         # 18 chunks per tile
SPT = CPT * P               # 2304 edge slots per tile
GROUP_TILES = 7
N_GROUPS = TPC // GROUP_TILES   # 14
SEG_IDXS = GROUP_TILES * SPTB   # 2688 idxs per (group, bank) gather
SEG_COLS = SEG_IDXS // 16       # 168
N_SEGS = N_GROUPS * N_BANKS     # 84
IDX_COLS = N_SEGS * SEG_COLS    # 14112

BF16 = ml_dtypes.bfloat16

LAST_RESULTS = None


# ================================================================ host prep
def _preprocess(x, edge_index):
    x = np.asarray(x, dtype=np.float32)
    ei = np.asarray(edge_index, dtype=np.int64)
    src = ei[0]
    dst = ei[1]

    # degree includes the self loop (appended by the reference)
    deg = (np.bincount(dst, minlength=N_NODES) + 1).astype(np.float64)
    dinv = 1.0 / np.sqrt(np.maximum(deg, 1e-12))

    # Self loops are NOT routed through the gather: their contribution is a
    # diagonal term handled by one identity-rhs matmul per tile.

    # ---- slot assignment: degree-stratified round robin (snake) over tiles
    order = np.argsort(-deg, kind="stable")
    k = np.arange(N_NODES)
    r = k // NT
    pos = k % NT
    tile_of_k = np.where(r % 2 == 0, pos, NT - 1 - pos)
    slot_of_node = np.empty(N_NODES, dtype=np.int64)
    slot_of_node[order] = tile_of_k * P + r

    # per-tile degree check
    tile_deg = np.bincount(slot_of_node[dst] // P, minlength=NT)
    assert tile_deg.max() <= SPT - 24, f"tile overload: {tile_deg.max()}"

    # ---- per-edge quantities
    eslot_dst = slot_of_node[dst]
    tile_e = (eslot_dst // P).astype(np.int64)
    dstl_e = (eslot_dst % P).astype(np.int32)
    sslot = slot_of_node[src].astype(np.int64)

    offs = np.asarray(BANK_OFF, dtype=np.int64)
    # allowed banks for edge e: lo_e..hi_e  (interval)
    lo_e = np.searchsorted(offs, sslot - (BANK_ROWS - 1), side="left")
    hi_e = np.searchsorted(offs, sslot, side="right") - 1
    assert (lo_e <= hi_e).all()

    # ---- per-tile EDF bank assignment with caps of SPTB real edges
    order_e = np.lexsort((hi_e, tile_e))   # by tile, then deadline
    t_sorted = tile_e[order_e]
    tile_starts = np.searchsorted(t_sorted, np.arange(NT + 1))

    # outputs
    gidx = np.zeros((N_CORES, P, IDX_COLS), dtype=np.int16)
    dstp = np.full((N_CORES, P, TPC * CPT), 200.0, dtype=np.float32)
    rng = np.random.RandomState(1234)

    flat_idx = np.empty((N_CORES, TPC, N_BANKS, SPTB), dtype=np.int16)
    flat_dstl = np.full((N_CORES, TPC, N_BANKS, SPTB), 200, dtype=np.int32)

    for t in range(NT):
        es = order_e[tile_starts[t]:tile_starts[t + 1]]   # edges, by hi asc
        elo = lo_e[es]
        ehi = hi_e[es]
        assigned = np.full(len(es), -1, dtype=np.int8)
        for b in range(N_BANKS):
            cand = np.nonzero((assigned == -1) & (elo <= b))[0]
            take = cand[:SPTB]
            assigned[take] = b
            left = (assigned == -1) & (ehi == b)
            if left.any():
                raise RuntimeError(f"bank overflow tile {t} bank {b}")
        assert (assigned >= 0).all()
        c = t // TPC
        tl = t % TPC
        for b in range(N_BANKS):
            sel = es[assigned == b]
            n = len(sel)
            assert n <= SPTB
            fi = flat_idx[c, tl, b]
            fd = flat_dstl[c, tl, b]
            fi[:n] = (sslot[sel] - BANK_OFF[b]).astype(np.int16)
            fd[:n] = dstl_e[sel]
            if n < SPTB:
                fi[n:] = rng.randint(0, BANK_ROWS, size=SPTB - n).astype(np.int16)
                # fd stays 200 (pad -> S row all zero)

    # sanity: idx in range
    assert flat_idx.min() >= 0

    # ---- pack gidx (wrapped 16, replicated to 128 partitions) and dstp
    for c in range(N_CORES):
        for g in range(N_GROUPS):
            for b in range(N_BANKS):
                seg = g * N_BANKS + b
                # concat the 7 tiles' (t,b) runs
                vals = flat_idx[c, g * GROUP_TILES:(g + 1) * GROUP_TILES, b].reshape(-1)
                w = vals.reshape(SEG_COLS, 16).T          # [16, SEG_COLS]
                gidx[c, :, seg * SEG_COLS:(seg + 1) * SEG_COLS] = np.tile(w, (8, 1))
        # dstp: col = tl*CPT + b*CPB + jc ; partition = j%128
        d = flat_dstl[c].reshape(TPC, N_BANKS, CPB, P)    # [tl, b, jc, p]
        d = d.transpose(3, 0, 1, 2).reshape(P, TPC * CPT)
        dstp[c] = d.astype(np.float32)

    # ---- node-feature table in slot order, pre-scaled by dinv (bf16)
    xt = np.zeros((NS, IN_DIM), dtype=BF16)
    xt[slot_of_node] = (x * dinv[:, None].astype(np.float32)).astype(BF16)

    # ---- per-core dinv (ACT scale) and rdinv (bias rank-1 lhsT)
    dinv_slots = np.zeros(NS, dtype=np.float32)
    dinv_slots[slot_of_node] = dinv.astype(np.float32)
    rdinv_slots = np.zeros(NS, dtype=np.float32)
    rdinv_slots[slot_of_node] = (1.0 / dinv).astype(np.float32)
    dinv_t = dinv_slots.reshape(N_CORES, TPC, P).transpose(0, 2, 1).copy()  # [c,128,98]
    # layer-1 ACT scale is dinv^2: it also folds the src-side dinv the
    # layer-2 gather needs into the h1 table (relu commutes with scale>0)
    dinv2_t = (dinv_t * dinv_t).astype(np.float32)
    rdinv_row = rdinv_slots.reshape(N_CORES, 1, SPC).astype(BF16)           # [c,1,12544]

    iota = np.tile(np.arange(P, dtype=np.float32).astype(BF16)[None, :], (P, 1))
    ident = np.eye(P, dtype=np.float32).astype(BF16)

    return dict(
        gidx=gidx, dstp=dstp, xt=xt, dinv_t=dinv_t, dinv2_t=dinv2_t,
        rdinv_row=rdinv_row, iota=iota, ident=ident,
        slot_of_node=slot_of_node,
    )


# ============================================================ numpy emulator
def _emulate(prep, W1, b1, W2, b2):
    """Numpy bit-for-bit-ish emulation of the device kernel (fp32 math on
    bf16-rounded data) to validate all the host-side layout logic."""
    xt = prep["xt"].astype(np.float32)
    gidx = prep["gidx"]
    dstp = prep["dstp"].astype(np.float32)
    dinv_t = prep["dinv_t"]
    rdinv = prep["rdinv_row"].astype(np.float32)
    w1 = W1.astype(BF16).astype(np.float32)
    w2 = W2.astype(BF16).astype(np.float32)
    b1f = b1.astype(BF16).astype(np.float32)
    b2f = b2.astype(BF16).astype(np.float32)

    def unwrap_seg(c, seg):
        w = gidx[c, :16, seg * SEG_COLS:(seg + 1) * SEG_COLS]
        return w.T.reshape(-1)   # [2688]

    def layer(table, w, bvec, relu, out_dim, scale_t):
        # table [NS, F] fp32 (already bf16-rounded values)
        h_out = np.zeros((N_CORES, SPC, out_dim), dtype=np.float32)
        F = table.shape[1]
        for c in range(N_CORES):
            for g in range(N_GROUPS):
                M = np.zeros((N_BANKS, GROUP_TILES * CPB, P, F), np.float32)
                for b in range(N_BANKS):
                    idxs = unwrap_seg(c, g * N_BANKS + b)
                    rows = table[BANK_OFF[b] + idxs.astype(np.int64)]
                    M[b] = rows.reshape(GROUP_TILES * CPB, P, F)
                for ti in range(GROUP_TILES):
                    tl = g * GROUP_TILES + ti
                    base = c * SPC + tl * P
                    # self-loop diagonal: psum[:, d] += table[base + d]
                    psum = table[base:base + P].astype(BF16).astype(np.float32).T.copy()
                    for cch in range(CPT):
                        b, j = divmod(cch, CPB)
                        mc = M[b, ti * CPB + j]            # [128e, F]
                        dcol = dstp[c, :, tl * CPT + cch]  # [128]
                        S = (dcol[:, None] == np.arange(P)[None, :]).astype(np.float32)
                        psum += mc.astype(BF16).astype(np.float32).T @ S
                    aggT = psum.astype(BF16).astype(np.float32)   # [F, 128d]
                    ps_b = aggT.T @ w                              # [128d, out]
                    u = rdinv[c, 0, tl * P:(tl + 1) * P]
                    ps_b = ps_b + u[:, None] * bvec[None, :]
                    scale = scale_t[c, :, tl]
                    o = ps_b * scale[:, None]
                    if relu:
                        o = np.maximum(o, 0.0)
                    h_out[c, tl * P:(tl + 1) * P] = o
        return h_out

    h1 = layer(xt, w1, b1f, True, HID_DIM, prep["dinv2_t"])
    h1_full = h1.reshape(NS, HID_DIM).astype(BF16).astype(np.float32)
    out = layer(h1_full, w2, b2f, False, OUT_DIM, dinv_t)
    return out.reshape(NS, OUT_DIM)[prep["slot_of_node"]]


# ============================================================= bass kernel
# The axon terminal cannot run ncfw collectives (NRT_EXEC_UNIT_UNRECOVERABLE),
# so the two GCN layers run as two NEFFs with a host-side h1 allgather.
_CACHED = {}


def _build_layer_nc(layer, reps=1):
    key = (layer, reps)
    if key in _CACHED:
        return _CACHED[key]

    import concourse.mybir as mybir
    import concourse.tile as tile
    from concourse import bacc, library_config

    f32 = mybir.dt.float32
    bf16 = mybir.dt.bfloat16
    i16 = mybir.dt.int16

    fdim = IN_DIM if layer == 1 else HID_DIM
    odim = HID_DIM if layer == 1 else OUT_DIM
    relu = layer == 1
    out_dt_np = BF16 if layer == 1 else np.float32

    nc = bacc.Bacc("TRN2", target_bir_lowering=False, debug=False,
                   num_devices=N_CORES, name=f"gcn_l{layer}r{reps}",
                   num_swdge_queues=4)

    tab_d = nc.dram_tensor("tab", [NS, fdim], bf16, kind="ExternalInput")
    self_d = nc.dram_tensor("selfb", [SPC, fdim], bf16, kind="ExternalInput")
    ident_d = nc.dram_tensor("ident", [P, P], bf16, kind="ExternalInput")
    gidx_d = nc.dram_tensor("gidx", [P, IDX_COLS], i16, kind="ExternalInput")
    dstp_d = nc.dram_tensor("dstp", [P, TPC * CPT], f32, kind="ExternalInput")
    dinv_d = nc.dram_tensor("dinv", [P, TPC], f32, kind="ExternalInput")
    rdinv_d = nc.dram_tensor("rdinv", [1, SPC], bf16, kind="ExternalInput")
    iota_d = nc.dram_tensor("iota", [P, P], bf16, kind="ExternalInput")
    w_d = nc.dram_tensor("w", [fdim, odim], bf16, kind="ExternalInput")
    b_d = nc.dram_tensor("b", [1, odim], bf16, kind="ExternalInput")
    out_d = nc.dram_tensor(
        "out", [SPC, odim],
        bf16 if layer == 1 else f32, kind="ExternalOutput")

    GC = GROUP_TILES * CPB          # chunks per bank region in a group (21)
    NCH = N_BANKS * GC              # chunks per group (126)
    ofunc = (mybir.ActivationFunctionType.Relu if relu
             else mybir.ActivationFunctionType.Copy)
    out_sb_dt = bf16 if layer == 1 else f32

    with tile.TileContext(nc) as tc:
        nc.gpsimd.load_library(library_config.mlp)

        with (
            tc.tile_pool(name="const", bufs=1) as constp,
            tc.tile_pool(name="mbuf", bufs=3) as mpool,
            tc.tile_pool(name="sbuf_s", bufs=2) as spool,
            tc.tile_pool(name="agg", bufs=3) as aggp,
            tc.tile_pool(name="outp", bufs=3) as outp,
            tc.tile_pool(name="psA", bufs=2, space="PSUM") as psA,
            tc.tile_pool(name="psB", bufs=2, space="PSUM") as psB,
        ):
            # ---- load constants
            gidx_sb = constp.tile([P, IDX_COLS], i16)
            nc.sync.dma_start(gidx_sb[:], gidx_d[:, :])
            dstp_sb = constp.tile([P, TPC * CPT], f32)
            nc.sync.dma_start(dstp_sb[:], dstp_d[:, :])
            dinv_sb = constp.tile([P, TPC], f32)
            nc.sync.dma_start(dinv_sb[:], dinv_d[:, :])
            rdinv_sb = constp.tile([1, SPC], bf16)
            nc.sync.dma_start(rdinv_sb[:], rdinv_d[:, :])
            iota_sb = constp.tile([P, P], bf16)
            nc.sync.dma_start(iota_sb[:], iota_d[:, :])
            ident_sb = constp.tile([P, P], bf16)
            nc.sync.dma_start(ident_sb[:], ident_d[:, :])
            w_sb = constp.tile([fdim, odim], bf16)
            nc.sync.dma_start(w_sb[:], w_d[:, :])
            b_sb = constp.tile([1, odim], bf16)
            nc.sync.dma_start(b_sb[:], b_d[:, :])

            for g in [gg for _ in range(reps) for gg in range(N_GROUPS)]:
                m_t = mpool.tile([P, NCH, fdim], bf16, tag="m")
                for b in range(N_BANKS):
                    seg = g * N_BANKS + b
                    # Round-robin across the 4 SWDGE queues: queue q's
                    # descriptors are generated by Q7 core pair (2q, 2q+1),
                    # so 4 gathers generate concurrently (the serial Q7
                    # descriptor build is the kernel's critical path).
                    nc.gpsimd.dma_gather(
                        out_ap=m_t[:, b * GC:(b + 1) * GC, :],
                        in_ap=tab_d[BANK_OFF[b]:BANK_OFF[b] + BANK_ROWS, :],
                        idxs_ap=gidx_sb[:, seg * SEG_COLS:(seg + 1) * SEG_COLS],
                        num_idxs=SEG_IDXS,
                        num_idxs_reg=SEG_IDXS,
                        elem_size=fdim,
                        single_packet=False,
                        queue_num=seg % 4,
                    )
                # contiguous block of this core's own rows (self loops)
                self_t = mpool.tile([P, GROUP_TILES, fdim], bf16, tag="self")
                nc.sync.dma_start(
                    self_t[:],
                    self_d[g * GROUP_TILES * P:(g + 1) * GROUP_TILES * P, :]
                    .rearrange("(t j) f -> j t f", j=P),
                )
                for ti in range(GROUP_TILES):
                    tl = g * GROUP_TILES + ti
                    s_t = spool.tile([P, CPT, P], bf16, tag="s")
                    for cch in range(CPT):
                        col = tl * CPT + cch
                        nc.vector.tensor_scalar(
                            s_t[:, cch, :], iota_sb[:],
                            dstp_sb[:, col:col + 1], None,
                            mybir.AluOpType.is_equal,
                        )
                    ps_a = psA.tile([P, P], f32, tag="psa")
                    nc.tensor.matmul(
                        ps_a[:], lhsT=self_t[:, ti, :], rhs=ident_sb[:],
                        start=True, stop=False)
                    for cch in range(CPT):
                        b, j = divmod(cch, CPB)
                        nc.tensor.matmul(
                            ps_a[:],
                            lhsT=m_t[:, b * GC + ti * CPB + j, :],
                            rhs=s_t[:, cch, :],
                            start=False, stop=(cch == CPT - 1),
                        )
                    aggT = aggp.tile([P, P], bf16, tag="agg")
                    nc.vector.tensor_copy(aggT[:], ps_a[:])
                    ps_b = psB.tile([P, odim], f32, tag="psb")
                    nc.tensor.matmul(
                        ps_b[:], lhsT=rdinv_sb[:, tl * P:(tl + 1) * P],
                        rhs=b_sb[:], start=True, stop=False)
                    nc.tensor.matmul(
                        ps_b[:], lhsT=aggT[:], rhs=w_sb[:],
                        start=False, stop=True)
                    o_t = outp.tile([P, odim], out_sb_dt, tag="o")
                    nc.scalar.activation(
                        o_t[:], ps_b[:], ofunc,
                        scale=dinv_sb[:, tl:tl + 1])
                    nc.sync.dma_start(
                        out_d[tl * P:(tl + 1) * P, :], o_t[:])

    nc.compile()
    _CACHED[key] = nc
    return nc


# ================================================================== kernel
def _run_layer(layer, table, W, b, prep, trace):
    from concourse.bass_utils import run_bass_kernel_spmd

    nc = _build_layer_nc(layer)
    base = {
        "tab": np.ascontiguousarray(table),
        "iota": np.ascontiguousarray(prep["iota"]),
        "ident": np.ascontiguousarray(prep["ident"]),
        "w": np.ascontiguousarray(np.asarray(W, np.float32).astype(BF16)),
        "b": np.ascontiguousarray(np.asarray(b, np.float32).astype(BF16)[None, :]),
    }
    in_maps = []
    for c in range(N_CORES):
        m = dict(base)
        m["selfb"] = np.ascontiguousarray(table[c * SPC:(c + 1) * SPC])
        m["gidx"] = np.ascontiguousarray(prep["gidx"][c])
        m["dstp"] = np.ascontiguousarray(prep["dstp"][c])
        m["dinv"] = np.ascontiguousarray(
            prep["dinv2_t"][c] if layer == 1 else prep["dinv_t"][c])
        m["rdinv"] = np.ascontiguousarray(prep["rdinv_row"][c])
        in_maps.append(m)
    res = run_bass_kernel_spmd(nc, in_maps, core_ids=list(range(N_CORES)),
                               trace=trace)
    return res, np.concatenate([r["out"] for r in res.results], axis=0)


def kernel(x, edge_index, W1, b1, W2, b2):
    prep = _preprocess(x, edge_index)
    trace = bool(os.environ.get("GCN_TRACE"))

    res1, h1full = _run_layer(1, prep["xt"], W1, b1, prep, trace)
    res2, big = _run_layer(2, h1full, W2, b2, prep, trace)

    global LAST_RESULTS
    LAST_RESULTS = (res1, res2)
    return np.ascontiguousarray(big[prep["slot_of_node"]]).astype(np.float32)



# revision 10
# speedup vs baseline: 2.4212x; 1.0632x over previous
"""GCN encoder (2-layer GCNConv) on 8 Trainium2 NeuronCores.

Strategy (pull model, dst-sharded):
  out = A @ relu(A @ x @ W1 + b1) @ W2 + b2,  A = D^-1/2 (Adj+I) D^-1/2
Reassociate: agg = A @ x first, then dense matmul by W (A@(xW) == (A@x)W).
Fold the src-side dinv into x on the host (x~ = dinv * x) and the dst-side
dinv into a per-partition ACT scale.  The sparse aggregation is done as
PE selection-matrix matmuls over edge chunks of 128:
  psum[feat, dst128] += M_chunk[e,feat].T @ S_chunk[e, dst128]
where M_chunk is dma_gather'ed rows of x~ (bf16) and S is a 0/1 matrix
built on DVE with is_equal(iota, dstl).

Host-side: nodes are dealt into 784 tiles of 128 slots (degree-stratified
round robin), edges are grouped by (dst tile, src bank) where banks are 6
overlapping 32768-row windows of the slot space (dma_gather indices are
int16).  Each (tile, bank) cell is padded to exactly 384 edge slots so one
static NEFF serves all 8 cores.  h1 is exchanged with an AllGather.
"""

import os

import numpy as np
import ml_dtypes

# ---------------------------------------------------------------- constants
N_NODES = 100000
N_EDGES = 1600000
IN_DIM = 128
HID_DIM = 128
OUT_DIM = 64
P = 128

N_CORES = 8
TPC = 98                    # tiles per core
SPC = TPC * P               # 12544 slots per core
NS = N_CORES * SPC          # 100352 slots total
NT = N_CORES * TPC          # 784 tiles total

N_BANKS = 6
BANK_ROWS = 32768
BANK_OFF = [0, 13517, 27034, 40551, 54068, 67584]
CPB = 3                     # chunks per (tile, bank)
SPTB = CPB * P              # 384 edge slots per (tile, bank)
CPT = N_BANKS * CPB         # 18 chunks per tile
SPT = CPT * P               # 2304 edge slots per tile
GROUP_TILES = 7
N_GROUPS = TPC // GROUP_TILES   # 14
SEG_IDXS = GROUP_TILES * SPTB   # 2688 idxs per (group, bank) gather
SEG_COLS = SEG_IDXS // 16       # 168
N_SEGS = N_GROUPS * N_BANKS     # 84
IDX_COLS = N_SEGS * SEG_COLS    # 14112

BF16 = ml_dtypes.bfloat16

LAST_RESULTS = None


# ================================================================ host prep
def _preprocess(x, edge_index):
    x = np.asarray(x, dtype=np.float32)
    ei = np.asarray(edge_index, dtype=np.int64)
    src = ei[0]
    dst = ei[1]

    # degree includes the self loop (appended by the reference)
    deg = (np.bincount(dst, minlength=N_NODES) + 1).astype(np.float64)
    dinv = 1.0 / np.sqrt(np.maximum(deg, 1e-12))

    # Self loops are NOT routed through the gather: their contribution is a
    # diagonal term handled by one identity-rhs matmul per tile.

    # ---- slot assignment: degree-stratified round robin (snake) over tiles
    order = np.argsort(-deg, kind="stable")
    k = np.arange(N_NODES)
    r = k // NT
    pos = k % NT
    tile_of_k = np.where(r % 2 == 0, pos, NT - 1 - pos)
    slot_of_node = np.empty(N_NODES, dtype=np.int64)
    slot_of_node[order] = tile_of_k * P + r

    # per-tile degree check
    tile_deg = np.bincount(slot_of_node[dst] // P, minlength=NT)
    assert tile_deg.max() <= SPT - 24, f"tile overload: {tile_deg.max()}"

    # ---- per-edge quantities
    eslot_dst = slot_of_node[dst]
    tile_e = (eslot_dst // P).astype(np.int64)
    dstl_e = (eslot_dst % P).astype(np.int32)
    sslot = slot_of_node[src].astype(np.int64)

    offs = np.asarray(BANK_OFF, dtype=np.int64)
    # allowed banks for edge e: lo_e..hi_e  (interval)
    lo_e = np.searchsorted(offs, sslot - (BANK_ROWS - 1), side="left")
    hi_e = np.searchsorted(offs, sslot, side="right") - 1
    assert (lo_e <= hi_e).all()

    # ---- per-tile EDF bank assignment with caps of SPTB real edges
    order_e = np.lexsort((hi_e, tile_e))   # by tile, then deadline
    t_sorted = tile_e[order_e]
    tile_starts = np.searchsorted(t_sorted, np.arange(NT + 1))

    # outputs
    gidx = np.zeros((N_CORES, P, IDX_COLS), dtype=np.int16)
    dstp = np.full((N_CORES, P, TPC * CPT), 200.0, dtype=np.float32)
    rng = np.random.RandomState(1234)

    flat_idx = np.empty((N_CORES, TPC, N_BANKS, SPTB), dtype=np.int16)
    flat_dstl = np.full((N_CORES, TPC, N_BANKS, SPTB), 200, dtype=np.int32)

    for t in range(NT):
        es = order_e[tile_starts[t]:tile_starts[t + 1]]   # edges, by hi asc
        elo = lo_e[es]
        ehi = hi_e[es]
        assigned = np.full(len(es), -1, dtype=np.int8)
        for b in range(N_BANKS):
            cand = np.nonzero((assigned == -1) & (elo <= b))[0]
            take = cand[:SPTB]
            assigned[take] = b
            left = (assigned == -1) & (ehi == b)
            if left.any():
                raise RuntimeError(f"bank overflow tile {t} bank {b}")
        assert (assigned >= 0).all()
        c = t // TPC
        tl = t % TPC
        for b in range(N_BANKS):
            sel = es[assigned == b]
            n = len(sel)
            assert n <= SPTB
            fi = flat_idx[c, tl, b]
            fd = flat_dstl[c, tl, b]
            fi[:n] = (sslot[sel] - BANK_OFF[b]).astype(np.int16)
            fd[:n] = dstl_e[sel]
            if n < SPTB:
                fi[n:] = rng.randint(0, BANK_ROWS, size=SPTB - n).astype(np.int16)
                # fd stays 200 (pad -> S row all zero)

    # sanity: idx in range
    assert flat_idx.min() >= 0

    # ---- pack gidx (wrapped 16, replicated to 128 partitions) and dstp
    for c in range(N_CORES):
        for g in range(N_GROUPS):
            for b in range(N_BANKS):
                seg = g * N_BANKS + b
                # concat the 7 tiles' (t,b) runs
                vals = flat_idx[c, g * GROUP_TILES:(g + 1) * GROUP_TILES, b].reshape(-1)
                w = vals.reshape(SEG_COLS, 16).T          # [16, SEG_COLS]
                gidx[c, :, seg * SEG_COLS:(seg + 1) * SEG_COLS] = np.tile(w, (8, 1))
        # dstp: col = tl*CPT + b*CPB + jc ; partition = j%128
        d = flat_dstl[c].reshape(TPC, N_BANKS, CPB, P)    # [tl, b, jc, p]
        d = d.transpose(3, 0, 1, 2).reshape(P, TPC * CPT)
        dstp[c] = d.astype(np.float32)

    # ---- node-feature table in slot order, pre-scaled by dinv (bf16)
    xt = np.zeros((NS, IN_DIM), dtype=BF16)
    xt[slot_of_node] = (x * dinv[:, None].astype(np.float32)).astype(BF16)

    # ---- per-core dinv (ACT scale) and rdinv (bias rank-1 lhsT)
    dinv_slots = np.zeros(NS, dtype=np.float32)
    dinv_slots[slot_of_node] = dinv.astype(np.float32)
    rdinv_slots = np.zeros(NS, dtype=np.float32)
    rdinv_slots[slot_of_node] = (1.0 / dinv).astype(np.float32)
    dinv_t = dinv_slots.reshape(N_CORES, TPC, P).transpose(0, 2, 1).copy()  # [c,128,98]
    # layer-1 ACT scale is dinv^2: it also folds the src-side dinv the
    # layer-2 gather needs into the h1 table (relu commutes with scale>0)
    dinv2_t = (dinv_t * dinv_t).astype(np.float32)
    rdinv_row = rdinv_slots.reshape(N_CORES, 1, SPC).astype(BF16)           # [c,1,12544]

    iota = np.tile(np.arange(P, dtype=np.float32).astype(BF16)[None, :], (P, 1))
    ident = np.eye(P, dtype=np.float32).astype(BF16)

    return dict(
        gidx=gidx, dstp=dstp, xt=xt, dinv_t=dinv_t, dinv2_t=dinv2_t,
        rdinv_row=rdinv_row, iota=iota, ident=ident,
        slot_of_node=slot_of_node,
    )


# ============================================================ numpy emulator
def _emulate(prep, W1, b1, W2, b2):
    """Numpy bit-for-bit-ish emulation of the device kernel (fp32 math on
    bf16-rounded data) to validate all the host-side layout logic."""
    xt = prep["xt"].astype(np.float32)
    gidx = prep["gidx"]
    dstp = prep["dstp"].astype(np.float32)
    dinv_t = prep["dinv_t"]
    rdinv = prep["rdinv_row"].astype(np.float32)
    w1 = W1.astype(BF16).astype(np.float32)
    w2 = W2.astype(BF16).astype(np.float32)
    b1f = b1.astype(BF16).astype(np.float32)
    b2f = b2.astype(BF16).astype(np.float32)

    def unwrap_seg(c, seg):
        w = gidx[c, :16, seg * SEG_COLS:(seg + 1) * SEG_COLS]
        return w.T.reshape(-1)   # [2688]

    def layer(table, w, bvec, relu, out_dim, scale_t):
        # table [NS, F] fp32 (already bf16-rounded values)
        h_out = np.zeros((N_CORES, SPC, out_dim), dtype=np.float32)
        F = table.shape[1]
        for c in range(N_CORES):
            for g in range(N_GROUPS):
                M = np.zeros((N_BANKS, GROUP_TILES * CPB, P, F), np.float32)
                for b in range(N_BANKS):
                    idxs = unwrap_seg(c, g * N_BANKS + b)
                    rows = table[BANK_OFF[b] + idxs.astype(np.int64)]
                    M[b] = rows.reshape(GROUP_TILES * CPB, P, F)
                for ti in range(GROUP_TILES):
                    tl = g * GROUP_TILES + ti
                    base = c * SPC + tl * P
                    # self-loop diagonal: psum[:, d] += table[base + d]
                    psum = table[base:base + P].astype(BF16).astype(np.float32).T.copy()
                    for cch in range(CPT):
                        b, j = divmod(cch, CPB)
                        mc = M[b, ti * CPB + j]            # [128e, F]
                        dcol = dstp[c, :, tl * CPT + cch]  # [128]
                        S = (dcol[:, None] == np.arange(P)[None, :]).astype(np.float32)
                        psum += mc.astype(BF16).astype(np.float32).T @ S
                    aggT = psum.astype(BF16).astype(np.float32)   # [F, 128d]
                    ps_b = aggT.T @ w                              # [128d, out]
                    u = rdinv[c, 0, tl * P:(tl + 1) * P]
                    ps_b = ps_b + u[:, None] * bvec[None, :]
                    scale = scale_t[c, :, tl]
                    o = ps_b * scale[:, None]
                    if relu:
                        o = np.maximum(o, 0.0)
                    h_out[c, tl * P:(tl + 1) * P] = o
        return h_out

    h1 = layer(xt, w1, b1f, True, HID_DIM, prep["dinv2_t"])
    h1_full = h1.reshape(NS, HID_DIM).astype(BF16).astype(np.float32)
    out = layer(h1_full, w2, b2f, False, OUT_DIM, dinv_t)
    return out.reshape(NS, OUT_DIM)[prep["slot_of_node"]]


# ============================================================= bass kernel
# The axon terminal cannot run ncfw collectives (NRT_EXEC_UNIT_UNRECOVERABLE),
# so the two GCN layers run as two NEFFs with a host-side h1 allgather.
_CACHED = {}


def _build_layer_nc(layer, reps=1):
    key = (layer, reps)
    if key in _CACHED:
        return _CACHED[key]

    import concourse.mybir as mybir
    import concourse.tile as tile
    from concourse import bacc, library_config

    f32 = mybir.dt.float32
    bf16 = mybir.dt.bfloat16
    i16 = mybir.dt.int16

    fdim = IN_DIM if layer == 1 else HID_DIM
    odim = HID_DIM if layer == 1 else OUT_DIM
    relu = layer == 1
    out_dt_np = BF16 if layer == 1 else np.float32

    nc = bacc.Bacc("TRN2", target_bir_lowering=False, debug=False,
                   num_devices=N_CORES, name=f"gcn_l{layer}r{reps}",
                   num_swdge_queues=4)

    tab_d = nc.dram_tensor("tab", [NS, fdim], bf16, kind="ExternalInput")
    self_d = nc.dram_tensor("selfb", [SPC, fdim], bf16, kind="ExternalInput")
    ident_d = nc.dram_tensor("ident", [P, P], bf16, kind="ExternalInput")
    gidx_d = nc.dram_tensor("gidx", [P, IDX_COLS], i16, kind="ExternalInput")
    dstp_d = nc.dram_tensor("dstp", [P, TPC * CPT], f32, kind="ExternalInput")
    dinv_d = nc.dram_tensor("dinv", [P, TPC], f32, kind="ExternalInput")
    rdinv_d = nc.dram_tensor("rdinv", [1, SPC], bf16, kind="ExternalInput")
    iota_d = nc.dram_tensor("iota", [P, P], bf16, kind="ExternalInput")
    w_d = nc.dram_tensor("w", [fdim, odim], bf16, kind="ExternalInput")
    b_d = nc.dram_tensor("b", [1, odim], bf16, kind="ExternalInput")
    out_d = nc.dram_tensor(
        "out", [SPC, odim],
        bf16 if layer == 1 else f32, kind="ExternalOutput")

    GC = GROUP_TILES * CPB          # chunks per bank region in a group (21)
    NCH = N_BANKS * GC              # chunks per group (126)
    ofunc = (mybir.ActivationFunctionType.Relu if relu
             else mybir.ActivationFunctionType.Copy)
    out_sb_dt = bf16 if layer == 1 else f32

    with tile.TileContext(nc) as tc:
        nc.gpsimd.load_library(library_config.mlp)

        with (
            tc.tile_pool(name="const", bufs=1) as constp,
            tc.tile_pool(name="mbuf", bufs=3) as mpool,
            tc.tile_pool(name="selfb", bufs=2) as selfp,
            tc.tile_pool(name="sbuf_s", bufs=8) as spool,
            tc.tile_pool(name="agg", bufs=3) as aggp,
            tc.tile_pool(name="outp", bufs=3) as outp,
            tc.tile_pool(name="psA", bufs=2, space="PSUM") as psA,
            tc.tile_pool(name="psB", bufs=2, space="PSUM") as psB,
        ):
            # ---- load constants
            gidx_sb = constp.tile([P, IDX_COLS], i16)
            nc.sync.dma_start(gidx_sb[:], gidx_d[:, :])
            dstp_sb = constp.tile([P, TPC * CPT], f32)
            nc.sync.dma_start(dstp_sb[:], dstp_d[:, :])
            dinv_sb = constp.tile([P, TPC], f32)
            nc.sync.dma_start(dinv_sb[:], dinv_d[:, :])
            rdinv_sb = constp.tile([1, SPC], bf16)
            nc.sync.dma_start(rdinv_sb[:], rdinv_d[:, :])
            iota_sb = constp.tile([P, P], bf16)
            nc.sync.dma_start(iota_sb[:], iota_d[:, :])
            ident_sb = constp.tile([P, P], bf16)
            nc.sync.dma_start(ident_sb[:], ident_d[:, :])
            w_sb = constp.tile([fdim, odim], bf16)
            nc.sync.dma_start(w_sb[:], w_d[:, :])
            b_sb = constp.tile([1, odim], bf16)
            nc.sync.dma_start(b_sb[:], b_d[:, :])

            # Halves of each (group, bank) segment, in matmul-chunk units.
            # Splitting every gather in two and round-robining the 4 SWDGE
            # queues keeps all four Q7 core pairs (queue q -> cores 2q,2q+1)
            # evenly loaded: descriptor generation is the critical path.
            HA_CH = 10                       # chunks in half A
            HA_IDX = HA_CH * P               # 1280 idxs
            HA_COLS = HA_IDX // 16           # 80 idx cols
            gctr = 0
            for g in [gg for _ in range(reps) for gg in range(N_GROUPS)]:
                m_t = mpool.tile([P, NCH, fdim], bf16, tag="m")
                for b in range(N_BANKS):
                    seg = g * N_BANKS + b
                    col0 = seg * SEG_COLS
                    for (c_lo, c_hi, i_lo, i_hi) in (
                        (0, HA_CH, 0, HA_COLS),
                        (HA_CH, GC, HA_COLS, SEG_COLS),
                    ):
                        n_idx = (c_hi - c_lo) * P
                        nc.gpsimd.dma_gather(
                            out_ap=m_t[:, b * GC + c_lo:b * GC + c_hi, :],
                            in_ap=tab_d[BANK_OFF[b]:BANK_OFF[b] + BANK_ROWS, :],
                            idxs_ap=gidx_sb[:, col0 + i_lo:col0 + i_hi],
                            num_idxs=n_idx,
                            num_idxs_reg=n_idx,
                            elem_size=fdim,
                            single_packet=False,
                            queue_num=gctr % 4,
                        )
                        gctr += 1
                # contiguous block of this core's own rows (self loops)
                self_t = selfp.tile([P, GROUP_TILES, fdim], bf16, tag="self")
                nc.sync.dma_start(
                    self_t[:],
                    self_d[g * GROUP_TILES * P:(g + 1) * GROUP_TILES * P, :]
                    .rearrange("(t j) f -> j t f", j=P),
                )
                # Prebuild the whole group's S selection matrices before the
                # matmul chains: the DVE stream then runs a group ahead of PE
                # instead of blocking behind each tile's psum copy.
                s_ts = []
                for ti in range(GROUP_TILES):
                    tl = g * GROUP_TILES + ti
                    s_t = spool.tile([P, CPT, P], bf16, tag="s")
                    for cch in range(CPT):
                        col = tl * CPT + cch
                        nc.vector.tensor_scalar(
                            s_t[:, cch, :], iota_sb[:],
                            dstp_sb[:, col:col + 1], None,
                            mybir.AluOpType.is_equal,
                        )
                    s_ts.append(s_t)
                for ti in range(GROUP_TILES):
                    tl = g * GROUP_TILES + ti
                    s_t = s_ts[ti]
                    ps_a = psA.tile([P, P], f32, tag="psa")
                    nc.tensor.matmul(
                        ps_a[:], lhsT=self_t[:, ti, :], rhs=ident_sb[:],
                        start=True, stop=False)
                    for cch in range(CPT):
                        b, j = divmod(cch, CPB)
                        nc.tensor.matmul(
                            ps_a[:],
                            lhsT=m_t[:, b * GC + ti * CPB + j, :],
                            rhs=s_t[:, cch, :],
                            start=False, stop=(cch == CPT - 1),
                        )
                    aggT = aggp.tile([P, P], bf16, tag="agg")
                    nc.vector.tensor_copy(aggT[:], ps_a[:])
                    ps_b = psB.tile([P, odim], f32, tag="psb")
                    nc.tensor.matmul(
                        ps_b[:], lhsT=rdinv_sb[:, tl * P:(tl + 1) * P],
                        rhs=b_sb[:], start=True, stop=False)
                    nc.tensor.matmul(
                        ps_b[:], lhsT=aggT[:], rhs=w_sb[:],
                        start=False, stop=True)
                    o_t = outp.tile([P, odim], out_sb_dt, tag="o")
                    nc.scalar.activation(
                        o_t[:], ps_b[:], ofunc,
                        scale=dinv_sb[:, tl:tl + 1])
                    nc.sync.dma_start(
                        out_d[tl * P:(tl + 1) * P, :], o_t[:])

    nc.compile()
    _CACHED[key] = nc
    return nc


# ================================================================== kernel
def _run_layer(layer, table, W, b, prep, trace):
    from concourse.bass_utils import run_bass_kernel_spmd

    nc = _build_layer_nc(layer)
    base = {
        "tab": np.ascontiguousarray(table),
        "iota": np.ascontiguousarray(prep["iota"]),
        "ident": np.ascontiguousarray(prep["ident"]),
        "w": np.ascontiguousarray(np.asarray(W, np.float32).astype(BF16)),
        "b": np.ascontiguousarray(np.asarray(b, np.float32).astype(BF16)[None, :]),
    }
    in_maps = []
    for c in range(N_CORES):
        m = dict(base)
        m["selfb"] = np.ascontiguousarray(table[c * SPC:(c + 1) * SPC])
        m["gidx"] = np.ascontiguousarray(prep["gidx"][c])
        m["dstp"] = np.ascontiguousarray(prep["dstp"][c])
        m["dinv"] = np.ascontiguousarray(
            prep["dinv2_t"][c] if layer == 1 else prep["dinv_t"][c])
        m["rdinv"] = np.ascontiguousarray(prep["rdinv_row"][c])
        in_maps.append(m)
    res = run_bass_kernel_spmd(nc, in_maps, core_ids=list(range(N_CORES)),
                               trace=trace)
    return res, np.concatenate([r["out"] for r in res.results], axis=0)


def kernel(x, edge_index, W1, b1, W2, b2):
    prep = _preprocess(x, edge_index)
    trace = bool(os.environ.get("GCN_TRACE"))

    res1, h1full = _run_layer(1, prep["xt"], W1, b1, prep, trace)
    res2, big = _run_layer(2, h1full, W2, b2, prep, trace)

    global LAST_RESULTS
    LAST_RESULTS = (res1, res2)
    return np.ascontiguousarray(big[prep["slot_of_node"]]).astype(np.float32)



# revision 13
# speedup vs baseline: 2.8783x; 1.1888x over previous
"""GCN encoder (2-layer GCNConv) on 8 Trainium2 NeuronCores.

Strategy (pull model, dst-sharded):
  out = A @ relu(A @ x @ W1 + b1) @ W2 + b2,  A = D^-1/2 (Adj+I) D^-1/2
Reassociate: agg = A @ x first, then dense matmul by W (A@(xW) == (A@x)W).
Fold the src-side dinv into x on the host (x~ = dinv * x) and the dst-side
dinv into a per-partition ACT scale.  The sparse aggregation is done as
PE selection-matrix matmuls over edge chunks of 128:
  psum[feat, dst128] += M_chunk[e,feat].T @ S_chunk[e, dst128]
where M_chunk is dma_gather'ed rows of x~ (bf16) and S is a 0/1 matrix
built on DVE with is_equal(iota, dstl).

Host-side: nodes are dealt into 784 tiles of 128 slots (degree-stratified
round robin), edges are grouped by (dst tile, src bank) where banks are 6
overlapping 32768-row windows of the slot space (dma_gather indices are
int16).  Each (tile, bank) cell is padded to exactly 384 edge slots so one
static NEFF serves all 8 cores.  h1 is exchanged with an AllGather.
"""

import os

import numpy as np
import ml_dtypes

# ---------------------------------------------------------------- constants
N_NODES = 100000
N_EDGES = 1600000
IN_DIM = 128
HID_DIM = 128
OUT_DIM = 64
P = 128

N_CORES = 8
TPC = 98                    # tiles per core
SPC = TPC * P               # 12544 slots per core
NS = N_CORES * SPC          # 100352 slots total
NT = N_CORES * TPC          # 784 tiles total

N_BANKS = 6
BANK_ROWS = 32768
BANK_OFF = [0, 13517, 27034, 40551, 54068, 67584]
CPB = 3                     # chunks per (tile, bank)
SPTB = CPB * P              # 384 edge slots per (tile, bank)
CPT = N_BANKS * CPB         # 18 chunks per tile
SPT = CPT * P               # 2304 edge slots per tile
GROUP_TILES = 7
N_GROUPS = TPC // GROUP_TILES   # 14
SEG_IDXS = GROUP_TILES * SPTB   # 2688 idxs per (group, bank) gather
SEG_COLS = SEG_IDXS // 16       # 168
N_SEGS = N_GROUPS * N_BANKS     # 84
IDX_COLS = N_SEGS * SEG_COLS    # 14112

BF16 = ml_dtypes.bfloat16

LAST_RESULTS = None


# ================================================================ host prep
def _preprocess(x, edge_index):
    x = np.asarray(x, dtype=np.float32)
    ei = np.asarray(edge_index, dtype=np.int64)
    src = ei[0]
    dst = ei[1]

    # degree includes the self loop (appended by the reference)
    deg = (np.bincount(dst, minlength=N_NODES) + 1).astype(np.float64)
    dinv = 1.0 / np.sqrt(np.maximum(deg, 1e-12))

    # Self loops are NOT routed through the gather: their contribution is a
    # diagonal term handled by one identity-rhs matmul per tile.

    # ---- slot assignment: degree-stratified round robin (snake) over tiles
    order = np.argsort(-deg, kind="stable")
    k = np.arange(N_NODES)
    r = k // NT
    pos = k % NT
    tile_of_k = np.where(r % 2 == 0, pos, NT - 1 - pos)
    slot_of_node = np.empty(N_NODES, dtype=np.int64)
    slot_of_node[order] = tile_of_k * P + r

    # per-tile degree check
    tile_deg = np.bincount(slot_of_node[dst] // P, minlength=NT)
    assert tile_deg.max() <= SPT - 24, f"tile overload: {tile_deg.max()}"

    # ---- per-edge quantities
    eslot_dst = slot_of_node[dst]
    tile_e = (eslot_dst // P).astype(np.int64)
    dstl_e = (eslot_dst % P).astype(np.int32)
    sslot = slot_of_node[src].astype(np.int64)

    offs = np.asarray(BANK_OFF, dtype=np.int64)
    # allowed banks for edge e: lo_e..hi_e  (interval)
    lo_e = np.searchsorted(offs, sslot - (BANK_ROWS - 1), side="left")
    hi_e = np.searchsorted(offs, sslot, side="right") - 1
    assert (lo_e <= hi_e).all()

    # ---- per-tile EDF bank assignment with caps of SPTB real edges
    order_e = np.lexsort((hi_e, tile_e))   # by tile, then deadline
    t_sorted = tile_e[order_e]
    tile_starts = np.searchsorted(t_sorted, np.arange(NT + 1))

    # outputs
    gidx = np.zeros((N_CORES, P, IDX_COLS), dtype=np.int16)
    dstp = np.full((N_CORES, P, TPC * CPT), 200.0, dtype=np.float32)
    rng = np.random.RandomState(1234)

    flat_idx = np.empty((N_CORES, TPC, N_BANKS, SPTB), dtype=np.int16)
    flat_dstl = np.full((N_CORES, TPC, N_BANKS, SPTB), 200, dtype=np.int32)

    for t in range(NT):
        es = order_e[tile_starts[t]:tile_starts[t + 1]]   # edges, by hi asc
        elo = lo_e[es]
        ehi = hi_e[es]
        assigned = np.full(len(es), -1, dtype=np.int8)
        for b in range(N_BANKS):
            cand = np.nonzero((assigned == -1) & (elo <= b))[0]
            take = cand[:SPTB]
            assigned[take] = b
            left = (assigned == -1) & (ehi == b)
            if left.any():
                raise RuntimeError(f"bank overflow tile {t} bank {b}")
        assert (assigned >= 0).all()
        c = t // TPC
        tl = t % TPC
        for b in range(N_BANKS):
            sel = es[assigned == b]
            n = len(sel)
            assert n <= SPTB
            fi = flat_idx[c, tl, b]
            fd = flat_dstl[c, tl, b]
            fi[:n] = (sslot[sel] - BANK_OFF[b]).astype(np.int16)
            fd[:n] = dstl_e[sel]
            if n < SPTB:
                fi[n:] = rng.randint(0, BANK_ROWS, size=SPTB - n).astype(np.int16)
                # fd stays 200 (pad -> S row all zero)

    # sanity: idx in range
    assert flat_idx.min() >= 0

    # ---- pack gidx (wrapped 16, replicated to 128 partitions) and dstp
    for c in range(N_CORES):
        for g in range(N_GROUPS):
            for b in range(N_BANKS):
                seg = g * N_BANKS + b
                # concat the 7 tiles' (t,b) runs
                vals = flat_idx[c, g * GROUP_TILES:(g + 1) * GROUP_TILES, b].reshape(-1)
                w = vals.reshape(SEG_COLS, 16).T          # [16, SEG_COLS]
                gidx[c, :, seg * SEG_COLS:(seg + 1) * SEG_COLS] = np.tile(w, (8, 1))
        # dstp: col = tl*CPT + b*CPB + jc ; partition = j%128
        d = flat_dstl[c].reshape(TPC, N_BANKS, CPB, P)    # [tl, b, jc, p]
        d = d.transpose(3, 0, 1, 2).reshape(P, TPC * CPT)
        dstp[c] = d.astype(np.float32)

    # ---- node-feature table in slot order, pre-scaled by dinv (bf16)
    xt = np.zeros((NS, IN_DIM), dtype=BF16)
    xt[slot_of_node] = (x * dinv[:, None].astype(np.float32)).astype(BF16)

    # ---- per-core dinv (ACT scale) and rdinv (bias rank-1 lhsT)
    dinv_slots = np.zeros(NS, dtype=np.float32)
    dinv_slots[slot_of_node] = dinv.astype(np.float32)
    rdinv_slots = np.zeros(NS, dtype=np.float32)
    rdinv_slots[slot_of_node] = (1.0 / dinv).astype(np.float32)
    dinv_t = dinv_slots.reshape(N_CORES, TPC, P).transpose(0, 2, 1).copy()  # [c,128,98]
    # layer-1 ACT scale is dinv^2: it also folds the src-side dinv the
    # layer-2 gather needs into the h1 table (relu commutes with scale>0)
    dinv2_t = (dinv_t * dinv_t).astype(np.float32)
    rdinv_row = rdinv_slots.reshape(N_CORES, 1, SPC).astype(BF16)           # [c,1,12544]

    iota = np.tile(np.arange(P, dtype=np.float32).astype(BF16)[None, :], (P, 1))
    ident = np.eye(P, dtype=np.float32).astype(BF16)

    return dict(
        gidx=gidx, dstp=dstp, xt=xt, dinv_t=dinv_t, dinv2_t=dinv2_t,
        rdinv_row=rdinv_row, iota=iota, ident=ident,
        slot_of_node=slot_of_node,
    )


# ============================================================ numpy emulator
def _emulate(prep, W1, b1, W2, b2):
    """Numpy bit-for-bit-ish emulation of the device kernel (fp32 math on
    bf16-rounded data) to validate all the host-side layout logic."""
    xt = prep["xt"].astype(np.float32)
    gidx = prep["gidx"]
    dstp = prep["dstp"].astype(np.float32)
    dinv_t = prep["dinv_t"]
    rdinv = prep["rdinv_row"].astype(np.float32)
    w1 = W1.astype(BF16).astype(np.float32)
    w2 = W2.astype(BF16).astype(np.float32)
    b1f = b1.astype(BF16).astype(np.float32)
    b2f = b2.astype(BF16).astype(np.float32)

    def unwrap_seg(c, seg):
        w = gidx[c, :16, seg * SEG_COLS:(seg + 1) * SEG_COLS]
        return w.T.reshape(-1)   # [2688]

    def layer(table, w, bvec, relu, out_dim, scale_t):
        # table [NS, F] fp32 (already bf16-rounded values)
        h_out = np.zeros((N_CORES, SPC, out_dim), dtype=np.float32)
        F = table.shape[1]
        for c in range(N_CORES):
            for g in range(N_GROUPS):
                M = np.zeros((N_BANKS, GROUP_TILES * CPB, P, F), np.float32)
                for b in range(N_BANKS):
                    idxs = unwrap_seg(c, g * N_BANKS + b)
                    rows = table[BANK_OFF[b] + idxs.astype(np.int64)]
                    M[b] = rows.reshape(GROUP_TILES * CPB, P, F)
                for ti in range(GROUP_TILES):
                    tl = g * GROUP_TILES + ti
                    base = c * SPC + tl * P
                    # self-loop diagonal: psum[:, d] += table[base + d]
                    psum = table[base:base + P].astype(BF16).astype(np.float32).T.copy()
                    for cch in range(CPT):
                        b, j = divmod(cch, CPB)
                        mc = M[b, ti * CPB + j]            # [128e, F]
                        dcol = dstp[c, :, tl * CPT + cch]  # [128]
                        S = (dcol[:, None] == np.arange(P)[None, :]).astype(np.float32)
                        psum += mc.astype(BF16).astype(np.float32).T @ S
                    aggT = psum.astype(BF16).astype(np.float32)   # [F, 128d]
                    ps_b = aggT.T @ w                              # [128d, out]
                    u = rdinv[c, 0, tl * P:(tl + 1) * P]
                    ps_b = ps_b + u[:, None] * bvec[None, :]
                    scale = scale_t[c, :, tl]
                    o = ps_b * scale[:, None]
                    if relu:
                        o = np.maximum(o, 0.0)
                    h_out[c, tl * P:(tl + 1) * P] = o
        return h_out

    h1 = layer(xt, w1, b1f, True, HID_DIM, prep["dinv2_t"])
    h1_full = h1.reshape(NS, HID_DIM).astype(BF16).astype(np.float32)
    out = layer(h1_full, w2, b2f, False, OUT_DIM, dinv_t)
    return out.reshape(NS, OUT_DIM)[prep["slot_of_node"]]


# ============================================================= bass kernel
# The axon terminal cannot run ncfw collectives (NRT_EXEC_UNIT_UNRECOVERABLE),
# so the two GCN layers run as two NEFFs with a host-side h1 allgather.
_CACHED = {}


def _build_layer_nc(layer, reps=1):
    key = (layer, reps)
    if key in _CACHED:
        return _CACHED[key]

    import concourse.mybir as mybir
    import concourse.tile as tile
    from concourse import bacc, library_config

    f32 = mybir.dt.float32
    bf16 = mybir.dt.bfloat16
    i16 = mybir.dt.int16

    fdim = IN_DIM if layer == 1 else HID_DIM
    odim = HID_DIM if layer == 1 else OUT_DIM
    relu = layer == 1
    out_dt_np = BF16 if layer == 1 else np.float32

    nc = bacc.Bacc("TRN2", target_bir_lowering=False, debug=False,
                   num_devices=N_CORES, name=f"gcn_l{layer}r{reps}",
                   num_swdge_queues=4)

    tab_d = nc.dram_tensor("tab", [NS, fdim], bf16, kind="ExternalInput")
    self_d = nc.dram_tensor("selfb", [SPC, fdim], bf16, kind="ExternalInput")
    ident_d = nc.dram_tensor("ident", [P, P], bf16, kind="ExternalInput")
    gidx_d = nc.dram_tensor("gidx", [P, IDX_COLS], i16, kind="ExternalInput")
    dstp_d = nc.dram_tensor("dstp", [P, TPC * CPT], f32, kind="ExternalInput")
    dinv_d = nc.dram_tensor("dinv", [P, TPC], f32, kind="ExternalInput")
    rdinv_d = nc.dram_tensor("rdinv", [1, SPC], bf16, kind="ExternalInput")
    iota_d = nc.dram_tensor("iota", [P, P], bf16, kind="ExternalInput")
    w_d = nc.dram_tensor("w", [fdim, odim], bf16, kind="ExternalInput")
    b_d = nc.dram_tensor("b", [1, odim], bf16, kind="ExternalInput")
    out_d = nc.dram_tensor(
        "out", [SPC, odim],
        bf16 if layer == 1 else f32, kind="ExternalOutput")

    GC = GROUP_TILES * CPB          # chunks per bank region in a group (21)
    NCH = N_BANKS * GC              # chunks per group (126)
    ofunc = (mybir.ActivationFunctionType.Relu if relu
             else mybir.ActivationFunctionType.Copy)
    out_sb_dt = bf16 if layer == 1 else f32

    with tile.TileContext(nc) as tc:
        nc.gpsimd.load_library(library_config.mlp)

        with (
            tc.tile_pool(name="const", bufs=1) as constp,
            tc.tile_pool(name="mbuf", bufs=3) as mpool,
            tc.tile_pool(name="selfb", bufs=2) as selfp,
            tc.tile_pool(name="sbuf_s", bufs=8) as spool,
            tc.tile_pool(name="agg", bufs=3) as aggp,
            tc.tile_pool(name="outp", bufs=3) as outp,
            tc.tile_pool(name="psA", bufs=2, space="PSUM") as psA,
            tc.tile_pool(name="psB", bufs=2, space="PSUM") as psB,
        ):
            # ---- load constants
            # Zero column for add-zero psum->sbuf copies on DVE.
            # tensor_tensor runs in single-port mode and never takes the
            # shared SBUF port pair; tensor_scalar / tensor_copy enter
            # 2-port perf mode, which locks GpSimd (SWDGE) out of SBUF and
            # serializes against gather descriptor generation.
            zero_sb = constp.tile([P, 1], f32)
            nc.vector.memset(zero_sb[:], 0.0)
            gidx_sb = constp.tile([P, IDX_COLS], i16)
            nc.sync.dma_start(gidx_sb[:], gidx_d[:, :])
            dstp_sb = constp.tile([P, TPC * CPT], f32)
            nc.sync.dma_start(dstp_sb[:], dstp_d[:, :])
            dinv_sb = constp.tile([P, TPC], f32)
            nc.sync.dma_start(dinv_sb[:], dinv_d[:, :])
            rdinv_sb = constp.tile([1, SPC], bf16)
            nc.sync.dma_start(rdinv_sb[:], rdinv_d[:, :])
            iota_sb = constp.tile([P, P], bf16)
            nc.sync.dma_start(iota_sb[:], iota_d[:, :])
            ident_sb = constp.tile([P, P], bf16)
            nc.sync.dma_start(ident_sb[:], ident_d[:, :])
            w_sb = constp.tile([fdim, odim], bf16)
            nc.sync.dma_start(w_sb[:], w_d[:, :])
            b_sb = constp.tile([1, odim], bf16)
            nc.sync.dma_start(b_sb[:], b_d[:, :])

            # Halves of each (group, bank) segment, in matmul-chunk units.
            # Splitting every gather in two and round-robining the 4 SWDGE
            # queues keeps all four Q7 core pairs (queue q -> cores 2q,2q+1)
            # evenly loaded: descriptor generation is the critical path.
            HA_CH = 10                       # chunks in half A
            HA_IDX = HA_CH * P               # 1280 idxs
            HA_COLS = HA_IDX // 16           # 80 idx cols
            gctr = 0
            for g in [gg for _ in range(reps) for gg in range(N_GROUPS)]:
                m_t = mpool.tile([P, NCH, fdim], bf16, tag="m")
                for b in range(N_BANKS):
                    seg = g * N_BANKS + b
                    col0 = seg * SEG_COLS
                    for (c_lo, c_hi, i_lo, i_hi) in (
                        (0, HA_CH, 0, HA_COLS),
                        (HA_CH, GC, HA_COLS, SEG_COLS),
                    ):
                        n_idx = (c_hi - c_lo) * P
                        nc.gpsimd.dma_gather(
                            out_ap=m_t[:, b * GC + c_lo:b * GC + c_hi, :],
                            in_ap=tab_d[BANK_OFF[b]:BANK_OFF[b] + BANK_ROWS, :],
                            idxs_ap=gidx_sb[:, col0 + i_lo:col0 + i_hi],
                            num_idxs=n_idx,
                            num_idxs_reg=n_idx,
                            elem_size=fdim,
                            single_packet=False,
                            queue_num=gctr % 4,
                        )
                        gctr += 1
                # contiguous block of this core's own rows (self loops)
                self_t = selfp.tile([P, GROUP_TILES, fdim], bf16, tag="self")
                nc.sync.dma_start(
                    self_t[:],
                    self_d[g * GROUP_TILES * P:(g + 1) * GROUP_TILES * P, :]
                    .rearrange("(t j) f -> j t f", j=P),
                )
                # Prebuild the whole group's S selection matrices before the
                # matmul chains: the DVE stream then runs a group ahead of PE
                # instead of blocking behind each tile's psum copy.
                s_ts = []
                for ti in range(GROUP_TILES):
                    tl = g * GROUP_TILES + ti
                    s_t = spool.tile([P, CPT, P], bf16, tag="s")
                    for cch in range(CPT):
                        col = tl * CPT + cch
                        nc.vector.tensor_tensor(
                            s_t[:, cch, :], iota_sb[:],
                            dstp_sb[:, col:col + 1].to_broadcast([P, P]),
                            mybir.AluOpType.is_equal,
                        )
                    s_ts.append(s_t)
                for ti in range(GROUP_TILES):
                    tl = g * GROUP_TILES + ti
                    s_t = s_ts[ti]
                    ps_a = psA.tile([P, P], f32, tag="psa")
                    nc.tensor.matmul(
                        ps_a[:], lhsT=self_t[:, ti, :], rhs=ident_sb[:],
                        start=True, stop=False)
                    for cch in range(CPT):
                        b, j = divmod(cch, CPB)
                        nc.tensor.matmul(
                            ps_a[:],
                            lhsT=m_t[:, b * GC + ti * CPB + j, :],
                            rhs=s_t[:, cch, :],
                            start=False, stop=(cch == CPT - 1),
                        )
                    aggT = aggp.tile([P, P], bf16, tag="agg")
                    nc.vector.tensor_tensor(
                        aggT[:], ps_a[:], zero_sb[:].to_broadcast([P, P]),
                        mybir.AluOpType.add)
                    ps_b = psB.tile([P, odim], f32, tag="psb")
                    nc.tensor.matmul(
                        ps_b[:], lhsT=rdinv_sb[:, tl * P:(tl + 1) * P],
                        rhs=b_sb[:], start=True, stop=False)
                    nc.tensor.matmul(
                        ps_b[:], lhsT=aggT[:], rhs=w_sb[:],
                        start=False, stop=True)
                    o_t = outp.tile([P, odim], out_sb_dt, tag="o")
                    nc.scalar.activation(
                        o_t[:], ps_b[:], ofunc,
                        scale=dinv_sb[:, tl:tl + 1])
                    nc.sync.dma_start(
                        out_d[tl * P:(tl + 1) * P, :], o_t[:])

    nc.compile()
    _CACHED[key] = nc
    return nc


# ================================================================== kernel
def _run_layer(layer, table, W, b, prep, trace):
    from concourse.bass_utils import run_bass_kernel_spmd

    nc = _build_layer_nc(layer)
    base = {
        "tab": np.ascontiguousarray(table),
        "iota": np.ascontiguousarray(prep["iota"]),
        "ident": np.ascontiguousarray(prep["ident"]),
        "w": np.ascontiguousarray(np.asarray(W, np.float32).astype(BF16)),
        "b": np.ascontiguousarray(np.asarray(b, np.float32).astype(BF16)[None, :]),
    }
    in_maps = []
    for c in range(N_CORES):
        m = dict(base)
        m["selfb"] = np.ascontiguousarray(table[c * SPC:(c + 1) * SPC])
        m["gidx"] = np.ascontiguousarray(prep["gidx"][c])
        m["dstp"] = np.ascontiguousarray(prep["dstp"][c])
        m["dinv"] = np.ascontiguousarray(
            prep["dinv2_t"][c] if layer == 1 else prep["dinv_t"][c])
        m["rdinv"] = np.ascontiguousarray(prep["rdinv_row"][c])
        in_maps.append(m)
    res = run_bass_kernel_spmd(nc, in_maps, core_ids=list(range(N_CORES)),
                               trace=trace)
    return res, np.concatenate([r["out"] for r in res.results], axis=0)


def kernel(x, edge_index, W1, b1, W2, b2):
    prep = _preprocess(x, edge_index)
    trace = bool(os.environ.get("GCN_TRACE"))

    res1, h1full = _run_layer(1, prep["xt"], W1, b1, prep, trace)
    res2, big = _run_layer(2, h1full, W2, b2, prep, trace)

    global LAST_RESULTS
    LAST_RESULTS = (res1, res2)
    return np.ascontiguousarray(big[prep["slot_of_node"]]).astype(np.float32)

